# revision 2
# baseline (speedup 1.0000x reference)
"""Trainium2 Bass kernel for nn_MultiHeadAttention_4810363372776 (linear attention).

Sharding: data-parallel over batch (4) x tensor-parallel over head groups (2).
Core i handles batch i//2, heads [8*(i%2), 8*(i%2)+8). Each core computes its
partial output projection; the host sums the two head-group partials per batch.
"""

import functools
import numpy as np

B, S, D, H = 4, 4096, 1024, 16
DK = D // H          # 64
OG = D // 2          # 512 per-core head-group width (8 heads)
NCORES = 8
SCALE = 1.0 / 8.0    # 1/sqrt(DK)
NT = S // 128        # 32 s-tiles


@functools.lru_cache(maxsize=2)
def _build(kv_bias=False):
    import concourse.bass as bass  # noqa: F401
    from concourse import bacc
    import concourse.mybir as mybir
    import concourse.tile as tile
    from concourse.masks import make_identity
    from contextlib import ExitStack

    f32 = mybir.dt.float32
    bf16 = mybir.dt.bfloat16
    fp8 = mybir.dt.float8e4
    DR = mybir.MatmulPerfMode.DoubleRow
    EXP = mybir.ActivationFunctionType.Exp
    COPY = mybir.ActivationFunctionType.Copy
    AXX = mybir.AxisListType.X
    ADD = mybir.AluOpType.add

    nc = bacc.Bacc()

    xq = nc.declare_dram_parameter("xq", [S, D], f32, isOutput=False)
    xk = nc.declare_dram_parameter("xk", [S, D], f32, isOutput=False)
    xv = nc.declare_dram_parameter("xv", [S, D], f32, isOutput=False)
    wqt = nc.declare_dram_parameter("wqt", [D, OG], fp8, isOutput=False)
    wkt = nc.declare_dram_parameter("wkt", [D, OG], fp8, isOutput=False)
    wvt = nc.declare_dram_parameter("wvt", [D, OG], bf16, isOutput=False)
    wot = nc.declare_dram_parameter("wot", [OG, D], bf16, isOutput=False)
    bqsp = nc.declare_dram_parameter("bqs", [128, 4], f32, isOutput=False)
    bkp = nc.declare_dram_parameter("bk", [1, OG], f32, isOutput=False)
    bvp = nc.declare_dram_parameter("bv", [1, OG], f32, isOutput=False)
    bop = nc.declare_dram_parameter("bo", [1, D], f32, isOutput=False)
    maskp = nc.declare_dram_parameter("maskf", [128, NT], f32, isOutput=False)
    out = nc.declare_dram_parameter("out", [S, D], f32, isOutput=True)

    with tile.TileContext(nc) as tc:
        with ExitStack() as ctx:
            singles = ctx.enter_context(tc.tile_pool(name="singles", bufs=1))

            ident = singles.tile([128, 128], bf16)
            make_identity(nc, ident)

            wq_sb = singles.tile([128, 8, OG], fp8, tag="wq")
            nc.sync.dma_start(out=wq_sb, in_=wqt[:, :].rearrange("(t p) o -> p t o", p=128))
            wk_sb = singles.tile([128, 8, OG], fp8, tag="wk")
            nc.sync.dma_start(out=wk_sb, in_=wkt[:, :].rearrange("(t p) o -> p t o", p=128))
            wv_sb = singles.tile([128, 8, OG], bf16, tag="wv")
            nc.sync.dma_start(out=wv_sb, in_=wvt[:, :].rearrange("(t p) o -> p t o", p=128))
            wo_sb = singles.tile([128, 4, D], bf16, tag="wo")
            nc.sync.dma_start(out=wo_sb, in_=wot[:, :].rearrange("(t p) o -> p t o", p=128))

            bqs_sb = singles.tile([128, 4], f32, tag="bqs")
            nc.sync.dma_start(out=bqs_sb, in_=bqsp[:, :])
            bo_bc = singles.tile([128, D], f32, tag="bo_bc")
            nc.gpsimd.dma_start(out=bo_bc, in_=bop[:, :].partition_broadcast(128))
            if kv_bias:
                bk_bc = singles.tile([128, OG], f32, tag="bk_bc")
                nc.gpsimd.dma_start(out=bk_bc, in_=bkp[:, :].partition_broadcast(128))
                bv_bc = singles.tile([128, OG], f32, tag="bv_bc")
                nc.gpsimd.dma_start(out=bv_bc, in_=bvp[:, :].partition_broadcast(128))
            mask_sb = singles.tile([128, NT], f32, tag="mask")
            nc.sync.dma_start(out=mask_sb, in_=maskp[:, :])

            # exp(q_hat * scale), stored [o (4 blocks of 128 = head pairs), s]
            ET = singles.tile([128, 4, S], bf16, tag="ET")
            # block-diag [kv | ksum] per head pair
            kvbd = [singles.tile([128, 130], bf16, tag=f"kvbd{p}", name=f"kvbd{p}") for p in range(4)]

            # ---------------- phase 1 ----------------
            SM = 512
            NU = SM // 128
            with ExitStack() as p1:
                pacc_pool = p1.enter_context(tc.tile_pool(name="pacc", bufs=1, space="PSUM"))
                # two chains per bank; bank-wide has_written clear happens once (st==0, even pair)
                kvps = [pacc_pool.tile([128, 2, 129], f32, tag=f"kvacc{i}", name=f"kvacc{i}") for i in range(2)]
                xin_pool = p1.enter_context(tc.tile_pool(name="xin", bufs=2))
                xt_pool = p1.enter_context(tc.tile_pool(name="xt", bufs=2))
                kvf_pool = p1.enter_context(tc.tile_pool(name="kvf", bufs=3))
                ptr_pool = p1.enter_context(tc.tile_pool(name="ptr", bufs=3, space="PSUM"))
                pkv_pool = p1.enter_context(tc.tile_pool(name="pkv", bufs=3, space="PSUM"))

                pending = None  # (kf, vf, st) deferred kv accumulation

                def flush_kv(pending):
                    kf, vf, pst = pending
                    for p in range(4):
                        nc.tensor.matmul(
                            kvps[p // 2][:, p % 2, 0:129],
                            kf[:, 2 * p:2 * p + 2, :],
                            vf[:, p, 0:129],
                            start=(pst == 0 and p % 2 == 0),
                            stop=(pst == NT - 1),
                            skip_group_check=True,
                        )

                for a in range(S // SM):
                    xq_sb = xin_pool.tile([128, NU, D], bf16, tag="xq")
                    nc.gpsimd.dma_start(out=xq_sb, in_=xq[a * SM:(a + 1) * SM, :].rearrange("(u p) d -> p u d", p=128))
                    xk_sb = xin_pool.tile([128, NU, D], bf16, tag="xk")
                    nc.gpsimd.dma_start(out=xk_sb, in_=xk[a * SM:(a + 1) * SM, :].rearrange("(u p) d -> p u d", p=128))
                    xv_sb = xin_pool.tile([128, NU, D], bf16, tag="xv")
                    nc.gpsimd.dma_start(out=xv_sb, in_=xv[a * SM:(a + 1) * SM, :].rearrange("(u p) d -> p u d", p=128))

                    xqT = xt_pool.tile([128, 8, SM], fp8, tag="xqT")
                    xkT = xt_pool.tile([128, 8, SM], fp8, tag="xkT")
                    xvT = xt_pool.tile([128, 8, SM], bf16, tag="xvT")

                    for u in range(NU):
                        st = a * NU + u

                        # transpose x tiles: [s,d] -> [d,s] via PE, evacuate to bf16
                        for x_sb, x_t, eng in (
                            (xq_sb, xqT, nc.vector),
                            (xk_sb, xkT, nc.scalar),
                            (xv_sb, xvT, nc.vector),
                        ):
                            for b2 in range(2):
                                ptr = ptr_pool.tile([128, 512], bf16, tag="tr")
                                for j in range(4):
                                    db = b2 * 4 + j
                                    nc.tensor.transpose(
                                        ptr[:, j * 128:(j + 1) * 128],
                                        x_sb[:, u, db * 128:(db + 1) * 128],
                                        ident,
                                    )
                                dst = x_t[:, b2 * 4:(b2 + 1) * 4, u * 128:(u + 1) * 128]
                                src = ptr.rearrange("p (j s) -> p j s", j=4)
                                if eng is nc.vector:
                                    nc.vector.tensor_copy(dst, src)
                                else:
                                    nc.scalar.copy(out=dst, in_=src)

                        # k projection
                        pk = pkv_pool.tile([128, OG], f32, tag="pkv")
                        for t2 in range(4):
                            nc.tensor.matmul(pk, xkT[:, 2 * t2:2 * t2 + 2, u * 128:(u + 1) * 128],
                                             wk_sb[:, 2 * t2:2 * t2 + 2, :],
                                             start=(t2 == 0), stop=(t2 == 3), perf_mode=DR)
                        if kv_bias:
                            nc.vector.tensor_add(pk, pk, bk_bc)
                        ek = kvf_pool.tile([128, OG], bf16, tag="ek")
                        nc.scalar.activation(ek, pk, EXP, scale=SCALE)
                        rows = kvf_pool.tile([128, 8], f32, tag="rows")
                        nc.vector.tensor_reduce(rows, ek.rearrange("p (h e) -> p h e", h=8), axis=AXX, op=ADD)
                        nc.vector.reciprocal(rows, rows)
                        nc.vector.tensor_scalar_mul(rows, rows, mask_sb[:, st:st + 1])
                        kf = kvf_pool.tile([128, 8, DK], bf16, tag="kf")
                        nc.vector.tensor_mul(
                            kf,
                            ek.rearrange("p (h e) -> p h e", h=8),
                            rows[:, :, None].to_broadcast([128, 8, DK]),
                        )

                        # v projection
                        pv = pkv_pool.tile([128, OG], f32, tag="pkv")
                        for t in range(8):
                            nc.tensor.matmul(pv, xvT[:, t, u * 128:(u + 1) * 128], wv_sb[:, t, :], start=(t == 0), stop=(t == 7))
                        if kv_bias:
                            nc.vector.tensor_add(pv, pv, bv_bc)
                        vf = kvf_pool.tile([128, 4, 130], bf16, tag="vf")
                        nc.scalar.activation(vf[:, :, 0:128], pv.rearrange("p (j s) -> p j s", j=4), COPY, scale=mask_sb[:, st:st + 1])
                        nc.vector.memset(vf[:, :, 128:129], 1.0)

                        # deferred kv accumulation for the previous s-tile
                        if pending is not None:
                            flush_kv(pending)
                        pending = (kf, vf, st)

                    # q projection for the macro, output transposed [o, s]
                    for ob in range(4):
                        pq = pkv_pool.tile([128, SM], f32, tag="pkv")
                        for t2 in range(4):
                            nc.tensor.matmul(pq, wq_sb[:, 2 * t2:2 * t2 + 2, ob * 128:(ob + 1) * 128],
                                             xqT[:, 2 * t2:2 * t2 + 2, :],
                                             start=(t2 == 0), stop=(t2 == 3), perf_mode=DR)
                        nc.scalar.activation(ET[:, ob, a * SM:(a + 1) * SM], pq, EXP, bias=bqs_sb[:, ob:ob + 1], scale=SCALE)

                flush_kv(pending)

                # build block-diag [kv | ksum] tiles (bf16)
                for p in range(4):
                    ps = kvps[p // 2][:, p % 2]
                    nc.vector.memset(kvbd[p], 0.0)
                    nc.vector.tensor_copy(kvbd[p][0:64, 0:64], ps[0:64, 0:64])
                    nc.vector.tensor_copy(kvbd[p][0:64, 64:65], ps[0:64, 128:129])
                    nc.vector.tensor_copy(kvbd[p][64:128, 65:129], ps[64:128, 64:128])
                    nc.vector.tensor_copy(kvbd[p][64:128, 129:130], ps[64:128, 128:129])

            # ---------------- phase 2 ----------------
            # stages per s-tile: num -> (DVE) ctx -> (PE) ctxT -> (ACT) evac -> (PE) out-proj
            # software-pipelined: ctxT lags one tile, out-proj lags two.
            with ExitStack() as p2s:
                p2 = p2s.enter_context(tc.tile_pool(name="p2", bufs=3))
                pnum_pool = p2s.enter_context(tc.tile_pool(name="pnum", bufs=2, space="PSUM"))
                pct_pool = p2s.enter_context(tc.tile_pool(name="pct", bufs=2, space="PSUM"))
                po_pool = p2s.enter_context(tc.tile_pool(name="po", bufs=2, space="PSUM"))

                ctx_q = {}   # st -> ctx tile
                ctxT_q = {}  # st -> ctxT tile

                def stage_num(st):
                    s0 = st * 128
                    pnums = [pnum_pool.tile([128, 2, 130], f32, tag=f"pnum{i}", name=f"pnum{i}") for i in range(2)]
                    for p in range(4):
                        nc.tensor.matmul(pnums[p // 2][:, p % 2, :], ET[:, p, s0:s0 + 128], kvbd[p], start=True, stop=True)
                    ctxs = p2.tile([128, OG], bf16, tag="ctx", name="ctxs")
                    for i in range(2):
                        pn4 = pnums[i].rearrange("p j (two c) -> p (j two) c", two=2)  # [128, 4, 65]
                        r4 = p2.tile([128, 4, 1], f32, tag="r", name="r4")
                        nc.vector.reciprocal(r4, pn4[:, :, 64:65])
                        ctx4 = ctxs[:, i * 256:(i + 1) * 256].rearrange("p (j c) -> p j c", c=64)
                        nc.vector.tensor_mul(ctx4, pn4[:, :, 0:64], r4.to_broadcast([128, 4, 64]))
                    ctx_q[st] = ctxs

                def stage_ctxT(st):
                    ctxs = ctx_q.pop(st)
                    pct = pct_pool.tile([128, 512], bf16, tag="pct", name="pct")
                    for eb in range(4):
                        nc.tensor.transpose(pct[:, eb * 128:(eb + 1) * 128], ctxs[:, eb * 128:(eb + 1) * 128], ident)
                    ctxT = p2.tile([128, 4, 128], bf16, tag="ctxT", name="ctxT")
                    nc.scalar.copy(out=ctxT, in_=pct.rearrange("p (j s) -> p j s", j=4))
                    ctxT_q[st] = ctxT

                def stage_oproj(st):
                    s0 = st * 128
                    ctxT = ctxT_q.pop(st)
                    outsb = p2.tile([128, D], f32, tag="outsb", name="outsb")
                    for half in range(2):
                        po = po_pool.tile([128, 512], f32, tag="po", name="po")
                        for eb in range(4):
                            nc.tensor.matmul(po, ctxT[:, eb, :], wo_sb[:, eb, half * 512:(half + 1) * 512], start=(eb == 0), stop=(eb == 3))
                        nc.vector.tensor_add(outsb[:, half * 512:(half + 1) * 512], po, bo_bc[:, half * 512:(half + 1) * 512])
                    nc.sync.dma_start(out=out[s0:s0 + 128, :], in_=outsb)

                for st in range(NT):
                    stage_num(st)
                    if st >= 1:
                        stage_ctxT(st - 1)
                    if st >= 2:
                        stage_oproj(st - 2)
                stage_ctxT(NT - 1)
                stage_oproj(NT - 2)
                stage_oproj(NT - 1)

    nc.compile()
    return nc


_LAST_RESULT = None


def _ensure_ntff_hook():
    """Make `antenv.axon_hooks` importable so BASS_TRACE profiling works.

    Some images ship a minimal `antenv` stub without `axon_hooks`; the boot
    shim then degrades silently and bass_utils crashes on import when
    trace=True under axon. Inject the module and install the ctypes NTFF
    hook if possible. No-op when the real module exists.
    """
    try:
        from antenv import axon_hooks  # noqa: F401
        return
    except ImportError:
        pass
    import sys
    import types
    try:
        import antenv
    except ImportError:
        return
    mod = types.ModuleType("antenv.axon_hooks")
    mod._hook = None

    def set_axon_ntff_profile_hook(hook):
        mod._hook = hook

    def get_axon_ntff_profile_hook():
        return mod._hook

    mod.set_axon_ntff_profile_hook = set_axon_ntff_profile_hook
    mod.get_axon_ntff_profile_hook = get_axon_ntff_profile_hook
    sys.modules["antenv.axon_hooks"] = mod
    antenv.axon_hooks = mod
    try:
        from trn_agent_boot.trn_boot import _ntff_profile_via_ctypes

        hook = _ntff_profile_via_ctypes("/opt/axon/libaxon_pjrt.so")
        if hook is not None:
            set_axon_ntff_profile_hook(hook)
    except Exception:
        pass


def kernel(q, k, v, mask, Wq, bq, Wk, bk, Wv, bv, Wo, bo):
    global _LAST_RESULT
    import ml_dtypes
    from concourse.bass_utils import run_bass_kernel_spmd

    _ensure_ntff_hook()

    q = np.asarray(q, np.float32)
    k = np.asarray(k, np.float32)
    v = np.asarray(v, np.float32)
    mask = np.asarray(mask)
    Wq = np.asarray(Wq, np.float32)
    Wk = np.asarray(Wk, np.float32)
    Wv = np.asarray(Wv, np.float32)
    Wo = np.asarray(Wo, np.float32)
    bq = np.asarray(bq, np.float32)
    bk = np.asarray(bk, np.float32)
    bv = np.asarray(bv, np.float32)
    bo = np.asarray(bo, np.float32)

    nc = _build(bool(np.any(bk) or np.any(bv)))

    bf = ml_dtypes.bfloat16
    f8 = ml_dtypes.float8_e4m3
    in_maps = []
    for core in range(NCORES):
        b, g = core // 2, core % 2
        sl = slice(g * OG, (g + 1) * OG)
        maskf = mask[b, 0, 0, :].astype(np.float32).reshape(NT, 128).T.copy()
        in_maps.append({
            "xq": np.ascontiguousarray(q[b]),
            "xk": np.ascontiguousarray(k[b]),
            "xv": np.ascontiguousarray(v[b]),
            "wqt": np.ascontiguousarray(Wq[sl, :].T).astype(f8),
            "wkt": np.ascontiguousarray(Wk[sl, :].T).astype(f8),
            "wvt": np.ascontiguousarray(Wv[sl, :].T).astype(bf),
            "wot": np.ascontiguousarray(Wo[:, sl].T).astype(bf),
            "bqs": np.ascontiguousarray((bq[sl] * SCALE).reshape(4, 128).T),
            "bk": bk[sl].reshape(1, OG).copy(),
            "bv": bv[sl].reshape(1, OG).copy(),
            "bo": (bo if g == 0 else np.zeros_like(bo)).reshape(1, D).copy(),
            "maskf": maskf,
        })

    res = run_bass_kernel_spmd(nc, in_maps, list(range(NCORES)))
    _LAST_RESULT = res

    outp = np.empty((B, S, D), np.float32)
    for b in range(B):
        outp[b] = res.results[2 * b]["out"] + res.results[2 * b + 1]["out"]
    return outp



# revision 9
# speedup vs baseline: 1.0085x; 1.0085x over previous
"""Trainium2 Bass kernel for nn_MultiHeadAttention_4810363372776 (linear attention).

Sharding: data-parallel over batch (4) x tensor-parallel over head groups (2).
Core i handles batch i//2, heads [8*(i%2), 8*(i%2)+8). Each core computes its
partial output projection; the host sums the two head-group partials per batch
and adds the output bias.

v2 layout: activations arrive pre-transposed ([d, s] tiled) and pre-cast
(q/k fp8, v bf16) from the host, so the kernel does zero PE transposes.
Phase 2 folds Wo through kv (M = kv_bd @ WoT): the output projection consumes
exp(q_hat) directly; per-head normalization is applied to exp(q_hat) via a
PSUM broadcast matmul (sel2) before the projection.
"""

import functools
import numpy as np

B, S, D, H = 4, 4096, 1024, 16
DK = D // H          # 64
OG = D // 2          # 512 per-core head-group width (8 heads)
NCORES = 8
SCALE = 1.0 / 8.0    # 1/sqrt(DK)
NT = S // 128        # 32 s-tiles
SM = 512             # s-chunk per input DMA
NCH = S // SM        # 8 chunks
NU = SM // 128       # tiles per chunk


@functools.lru_cache(maxsize=2)
def _build(kv_bias=False):
    import concourse.bass as bass  # noqa: F401
    from concourse import bacc
    import concourse.mybir as mybir
    import concourse.tile as tile
    from contextlib import ExitStack

    f32 = mybir.dt.float32
    bf16 = mybir.dt.bfloat16
    fp8 = mybir.dt.float8e4
    DR = mybir.MatmulPerfMode.DoubleRow
    EXP = mybir.ActivationFunctionType.Exp
    COPY = mybir.ActivationFunctionType.Copy
    AXX = mybir.AxisListType.X
    ADD = mybir.AluOpType.add

    nc = bacc.Bacc()

    # x*[p, a, t, s'] = x[SM*a + s', 128*t + p] — per-partition-contiguous chunks
    xq = nc.declare_dram_parameter("xq", [128, NCH, 8, SM], fp8, isOutput=False)
    xk = nc.declare_dram_parameter("xk", [128, NCH, 8, SM], fp8, isOutput=False)
    xv = nc.declare_dram_parameter("xv", [128, NCH, 8, SM], bf16, isOutput=False)
    # w*[p, t, o] = W[og_slice, :].T[128*t + p, o]
    wq = nc.declare_dram_parameter("wq", [128, 8, OG], fp8, isOutput=False)
    wk = nc.declare_dram_parameter("wk", [128, 8, OG], fp8, isOutput=False)
    wv = nc.declare_dram_parameter("wv", [128, 8, OG], bf16, isOutput=False)
    # wo[p, t, d] = Wo[:, og_slice].T[128*t + p, d]
    wo = nc.declare_dram_parameter("wo", [128, 4, D], bf16, isOutput=False)
    bqsp = nc.declare_dram_parameter("bqs", [128, 4], f32, isOutput=False)
    bkp = nc.declare_dram_parameter("bk", [1, OG], f32, isOutput=False)
    bvp = nc.declare_dram_parameter("bv", [1, OG], f32, isOutput=False)
    maskp = nc.declare_dram_parameter("maskf", [128, NT], f32, isOutput=False)
    sel2p = nc.declare_dram_parameter("sel2", [2, 128], bf16, isOutput=False)
    out = nc.declare_dram_parameter("out", [S, D], bf16, isOutput=True)

    with tile.TileContext(nc) as tc:
        with ExitStack() as ctx:
            singles = ctx.enter_context(tc.tile_pool(name="singles", bufs=1))

            wq_sb = singles.tile([128, 8, OG], fp8, tag="wq")
            nc.sync.dma_start(out=wq_sb, in_=wq[:, :, :])
            wk_sb = singles.tile([128, 8, OG], fp8, tag="wk")
            nc.sync.dma_start(out=wk_sb, in_=wk[:, :, :])
            wv_sb = singles.tile([128, 8, OG], bf16, tag="wv")
            nc.sync.dma_start(out=wv_sb, in_=wv[:, :, :])
            wo_sb = singles.tile([128, 4, D], bf16, tag="wo")
            nc.sync.dma_start(out=wo_sb, in_=wo[:, :, :])

            bqs_sb = singles.tile([128, 4], f32, tag="bqs")
            nc.sync.dma_start(out=bqs_sb, in_=bqsp[:, :])
            if kv_bias:
                bk_bc = singles.tile([128, OG], f32, tag="bk_bc")
                nc.gpsimd.dma_start(out=bk_bc, in_=bkp[:, :].partition_broadcast(128))
                bv_bc = singles.tile([128, OG], f32, tag="bv_bc")
                nc.gpsimd.dma_start(out=bv_bc, in_=bvp[:, :].partition_broadcast(128))
            mask_sb = singles.tile([128, NT], f32, tag="mask")
            nc.sync.dma_start(out=mask_sb, in_=maskp[:, :])

            # constants
            ones_s = singles.tile([128, 1], bf16, tag="ones_s")
            nc.vector.memset(ones_s, 1.0)
            ones1 = singles.tile([1, 1], f32, tag="ones1")
            nc.vector.memset(ones1, 1.0)
            # sel2: partition j broadcasts to o-rows of head j within a pair
            sel2 = singles.tile([2, 128], bf16, tag="sel2")
            nc.sync.dma_start(out=sel2, in_=sel2p[:, :])

            # exp(q_hat * scale), stored [o (4 blocks of 128 = head pairs), s]
            ET = singles.tile([128, 4, S], bf16, tag="ET")
            # fused (block-diag kv) @ WoT, plus per-pair block-diag ksum
            M_sb = singles.tile([128, 4, D], bf16, tag="M")
            ksum_bd = singles.tile([128, 4, 2], bf16, tag="ksum_bd")

            # ---------------- phase 1 ----------------
            with ExitStack() as p1:
                pacc_pool = p1.enter_context(tc.tile_pool(name="pacc", bufs=1, space="PSUM"))
                # kv^T accumulated per head pair: [e', pair, o'] (with cross-head
                # garbage at off-diagonal 64-blocks, masked out later)
                kvT_ps = pacc_pool.tile([128, 4, 128], f32, tag="kvT", name="kvT")
                # ksum^T accumulated per pair: [1, pair, o']
                ksT_ps = pacc_pool.tile([1, 4, 128], f32, tag="ksT", name="ksT")

                with ExitStack() as p1a:
                    xin_pool = p1a.enter_context(tc.tile_pool(name="xin", bufs=2))
                    kvf_pool = p1a.enter_context(tc.tile_pool(name="kvf", bufs=3))
                    pkv_pool = p1a.enter_context(tc.tile_pool(name="pkv", bufs=3, space="PSUM"))

                    pending = None  # (kf, vf, st) deferred kv accumulation

                    def flush_kv(pending):
                        kf, vf, pst = pending
                        for p in range(4):
                            nc.tensor.matmul(
                                kvT_ps[:, p, :],
                                vf[:, p, :],
                                kf[:, 2 * p:2 * p + 2, :],
                                start=(pst == 0 and p == 0),
                                stop=(pst == NT - 1),
                                skip_group_check=True,
                            )
                        for p in range(4):
                            nc.tensor.matmul(
                                ksT_ps[0:1, p, :],
                                ones_s,
                                kf[:, 2 * p:2 * p + 2, :],
                                start=(pst == 0 and p == 0),
                                stop=(pst == NT - 1),
                                skip_group_check=True,
                            )

                    for a in range(NCH):
                        xq_sb = xin_pool.tile([128, 8, SM], fp8, tag="xq")
                        nc.gpsimd.dma_start(out=xq_sb, in_=xq[:, a, :, :])
                        xk_sb = xin_pool.tile([128, 8, SM], fp8, tag="xk")
                        nc.gpsimd.dma_start(out=xk_sb, in_=xk[:, a, :, :])
                        xv_sb = xin_pool.tile([128, 8, SM], bf16, tag="xv")
                        nc.gpsimd.dma_start(out=xv_sb, in_=xv[:, a, :, :])

                        for u in range(NU):
                            st = a * NU + u
                            usl = slice(u * 128, (u + 1) * 128)

                            # k projection -> [s, og]
                            pk = pkv_pool.tile([128, OG], f32, tag="pkv")
                            for t2 in range(4):
                                nc.tensor.matmul(pk, xk_sb[:, 2 * t2:2 * t2 + 2, usl],
                                                 wk_sb[:, 2 * t2:2 * t2 + 2, :],
                                                 start=(t2 == 0), stop=(t2 == 3), perf_mode=DR)
                            if kv_bias:
                                nc.vector.tensor_add(pk, pk, bk_bc)
                            ek = kvf_pool.tile([128, OG], bf16, tag="ek")
                            nc.scalar.activation(ek, pk, EXP, scale=SCALE)
                            rows = kvf_pool.tile([128, 8], f32, tag="rows")
                            nc.vector.tensor_reduce(rows, ek.rearrange("p (h e) -> p h e", h=8), axis=AXX, op=ADD)
                            nc.vector.reciprocal(rows, rows)
                            nc.vector.tensor_scalar_mul(rows, rows, mask_sb[:, st:st + 1])
                            kf = kvf_pool.tile([128, 8, DK], bf16, tag="kf")
                            nc.vector.tensor_mul(
                                kf,
                                ek.rearrange("p (h e) -> p h e", h=8),
                                rows[:, :, None].to_broadcast([128, 8, DK]),
                            )

                            # v projection -> [s, og]
                            pv = pkv_pool.tile([128, OG], f32, tag="pkv")
                            for t in range(8):
                                nc.tensor.matmul(pv, xv_sb[:, t, usl], wv_sb[:, t, :],
                                                 start=(t == 0), stop=(t == 7))
                            if kv_bias:
                                nc.vector.tensor_add(pv, pv, bv_bc)
                            vf = kvf_pool.tile([128, 4, 128], bf16, tag="vf")
                            nc.scalar.activation(vf, pv.rearrange("p (j e) -> p j e", j=4),
                                                 COPY, scale=mask_sb[:, st:st + 1])

                            # deferred kv/ksum accumulation for the previous s-tile
                            if pending is not None:
                                flush_kv(pending)
                            pending = (kf, vf, st)

                        # q projection for the chunk, output transposed [o, s]
                        for ob in range(4):
                            pq = pkv_pool.tile([128, SM], f32, tag="pkv")
                            for t2 in range(4):
                                nc.tensor.matmul(pq, wq_sb[:, 2 * t2:2 * t2 + 2, ob * 128:(ob + 1) * 128],
                                                 xq_sb[:, 2 * t2:2 * t2 + 2, :],
                                                 start=(t2 == 0), stop=(t2 == 3), perf_mode=DR)
                            nc.scalar.activation(ET[:, ob, a * SM:(a + 1) * SM], pq, EXP,
                                                 bias=bqs_sb[:, ob:ob + 1], scale=SCALE)

                    flush_kv(pending)

                # ---------------- interphase ----------------
                pks_pool = p1.enter_context(tc.tile_pool(name="pks", bufs=1, space="PSUM"))
                pm_pool = p1.enter_context(tc.tile_pool(name="pm", bufs=2, space="PSUM"))

                # block-diagonal kv^T (zero the cross-head 64-blocks)
                kvbd = singles.tile([128, 4, 128], bf16, tag="kvbd")
                nc.vector.memset(kvbd, 0.0)
                for p in range(4):
                    nc.vector.tensor_copy(kvbd[0:64, p, 0:64], kvT_ps[0:64, p, 0:64])
                    nc.vector.tensor_copy(kvbd[64:128, p, 64:128], kvT_ps[64:128, p, 64:128])

                # transpose ksum^T [1, o'] -> [o', 1] via K=1 matmul, then block-diag
                ksT_sb = singles.tile([1, 4, 128], f32, tag="ksT_sb")
                nc.scalar.copy(out=ksT_sb, in_=ksT_ps)
                pks = pks_pool.tile([128, 4, 1], f32, tag="pks")
                for p in range(4):
                    nc.tensor.matmul(pks[:, p, :], ksT_sb[0:1, p, :], ones1, start=True, stop=True)
                nc.vector.memset(ksum_bd, 0.0)
                for p in range(4):
                    nc.vector.tensor_copy(ksum_bd[0:64, p, 0:1], pks[0:64, p, :])
                    nc.vector.tensor_copy(ksum_bd[64:128, p, 1:2], pks[64:128, p, :])

                # M = kv_bd @ WoT (per pair block)
                for p in range(4):
                    for half in range(2):
                        pm = pm_pool.tile([128, 512], f32, tag="pm")
                        nc.tensor.matmul(pm, kvbd[:, p, :], wo_sb[:, p, half * 512:(half + 1) * 512],
                                         start=True, stop=True)
                        nc.scalar.copy(out=M_sb[:, p, half * 512:(half + 1) * 512], in_=pm)

            # ---------------- phase 2 ----------------
            with ExitStack() as p2s:
                sb2 = p2s.enter_context(tc.tile_pool(name="sb2", bufs=3))
                pd_pool = p2s.enter_context(tc.tile_pool(name="pd", bufs=2, space="PSUM"))
                prd_pool = p2s.enter_context(tc.tile_pool(name="prd", bufs=2, space="PSUM"))
                pout_pool = p2s.enter_context(tc.tile_pool(name="pout", bufs=2, space="PSUM"))

                for st in range(NT):
                    s0 = st * 128
                    # per-head denominators: pd[j, p, s'] = <ksum_h, ET_h[:, s']>
                    pd = pd_pool.tile([2, 4, 128], f32, tag="pd", name="pd")
                    for p in range(4):
                        nc.tensor.matmul(pd[:, p, :], ksum_bd[:, p, :], ET[:, p, s0:s0 + 128],
                                         start=True, stop=True)
                    rs_pre = sb2.tile([2, 4, 128], f32, tag="rspre", name="rspre")
                    nc.scalar.activation(rs_pre, pd, COPY, bias=1e-6)
                    rs = sb2.tile([2, 4, 128], bf16, tag="rs", name="rs")
                    with nc.allow_low_precision(reason="1/denom in bf16 is within tolerance"):
                        nc.vector.reciprocal(rs, rs_pre)
                    # broadcast 1/denom to the 64 o-rows of each head
                    prd = prd_pool.tile([128, 4, 128], f32, tag="prd", name="prd")
                    nc.tensor.matmul(prd, sel2, rs, start=True, stop=True)
                    # normalized exp(q) features
                    ets = sb2.tile([128, 4, 128], bf16, tag="ets", name="ets")
                    nc.vector.tensor_mul(ets, ET[:, :, s0:s0 + 128], prd)
                    # fused output projection
                    pout = pout_pool.tile([128, 2, 512], f32, tag="pout", name="pout")
                    for half in range(2):
                        for p in range(4):
                            nc.tensor.matmul(pout[:, half, :], ets[:, p, :],
                                             M_sb[:, p, half * 512:(half + 1) * 512],
                                             start=(p == 0), stop=(p == 3))
                    outsb = sb2.tile([128, D], bf16, tag="outsb", name="outsb")
                    nc.scalar.copy(out=outsb.rearrange("p (j e) -> p j e", j=2), in_=pout)
                    nc.sync.dma_start(out=out[s0:s0 + 128, :], in_=outsb)

    nc.compile()
    return nc


_LAST_RESULT = None


def _ensure_ntff_hook():
    """Make `antenv.axon_hooks` importable so BASS_TRACE profiling works.

    Some images ship a minimal `antenv` stub without `axon_hooks`; the boot
    shim then degrades silently and bass_utils crashes on import when
    trace=True under axon. Inject the module and install the ctypes NTFF
    hook if possible. No-op when the real module exists.
    """
    try:
        from antenv import axon_hooks  # noqa: F401
        return
    except ImportError:
        pass
    import sys
    import types
    try:
        import antenv
    except ImportError:
        return
    mod = types.ModuleType("antenv.axon_hooks")
    mod._hook = None

    def set_axon_ntff_profile_hook(hook):
        mod._hook = hook

    def get_axon_ntff_profile_hook():
        return mod._hook

    mod.set_axon_ntff_profile_hook = set_axon_ntff_profile_hook
    mod.get_axon_ntff_profile_hook = get_axon_ntff_profile_hook
    sys.modules["antenv.axon_hooks"] = mod
    antenv.axon_hooks = mod
    try:
        from trn_agent_boot.trn_boot import _ntff_profile_via_ctypes

        hook = _ntff_profile_via_ctypes("/opt/axon/libaxon_pjrt.so")
        if hook is not None:
            set_axon_ntff_profile_hook(hook)
    except Exception:
        pass


def kernel(q, k, v, mask, Wq, bq, Wk, bk, Wv, bv, Wo, bo):
    global _LAST_RESULT
    import ml_dtypes
    from concourse.bass_utils import run_bass_kernel_spmd

    _ensure_ntff_hook()

    q = np.asarray(q, np.float32)
    k = np.asarray(k, np.float32)
    v = np.asarray(v, np.float32)
    mask = np.asarray(mask)
    Wq = np.asarray(Wq, np.float32)
    Wk = np.asarray(Wk, np.float32)
    Wv = np.asarray(Wv, np.float32)
    Wo = np.asarray(Wo, np.float32)
    bq = np.asarray(bq, np.float32)
    bk = np.asarray(bk, np.float32)
    bv = np.asarray(bv, np.float32)
    bo = np.asarray(bo, np.float32)

    nc = _build(bool(np.any(bk) or np.any(bv)))

    bf = ml_dtypes.bfloat16
    f8 = ml_dtypes.float8_e4m3

    def xtile(x, dt):
        # [S, D] -> [128, NCH, 8, SM]: A[p, a, t, s'] = x[SM*a + s', 128*t + p]
        xt = np.ascontiguousarray(x.T)
        return xt.reshape(8, 128, NCH, SM).transpose(1, 2, 0, 3).astype(dt)

    def wtile(W, sl, nt, dt):
        # [128, nt, ncols]: w[p, t, o] = W[sl, :].T[128*t + p, o]
        wt = np.ascontiguousarray(W[sl, :].T) if sl is not None else W
        return wt.reshape(nt, 128, -1).transpose(1, 0, 2).astype(dt)

    sel2_host = np.zeros((2, 128), bf)
    sel2_host[0, 0:64] = 1
    sel2_host[1, 64:128] = 1

    in_maps = []
    xcache = {}
    for core in range(NCORES):
        b, g = core // 2, core % 2
        sl = slice(g * OG, (g + 1) * OG)
        if b not in xcache:
            xcache[b] = (xtile(q[b], f8), xtile(k[b], f8), xtile(v[b], bf))
        xqh, xkh, xvh = xcache[b]
        maskf = mask[b, 0, 0, :].astype(np.float32).reshape(NT, 128).T.copy()
        in_maps.append({
            "xq": xqh,
            "xk": xkh,
            "xv": xvh,
            "wq": wtile(Wq, sl, 8, f8),
            "wk": wtile(Wk, sl, 8, f8),
            "wv": wtile(Wv, sl, 8, bf),
            "wo": wtile(np.ascontiguousarray(Wo[:, sl].T), None, 4, bf),
            "bqs": np.ascontiguousarray((bq[sl] * SCALE).reshape(4, 128).T),
            "bk": bk[sl].reshape(1, OG).copy(),
            "bv": bv[sl].reshape(1, OG).copy(),
            "maskf": maskf,
            "sel2": sel2_host,
        })

    res = run_bass_kernel_spmd(nc, in_maps, list(range(NCORES)))
    _LAST_RESULT = res

    outp = np.empty((B, S, D), np.float32)
    for b in range(B):
        outp[b] = (res.results[2 * b]["out"].astype(np.float32)
                   + res.results[2 * b + 1]["out"].astype(np.float32)
                   + bo[None, :])
    return outp


# revision 18
# speedup vs baseline: 1.2075x; 1.1973x over previous
"""Trainium2 Bass kernel for nn_MultiHeadAttention_4810363372776 (linear attention).

Sharding: data-parallel over batch (4) x tensor-parallel over head groups (2).
Core i handles batch i//2, heads [8*(i%2), 8*(i%2)+8). Each core computes its
partial output projection; the host sums the two head-group partials per batch
and adds the output bias.

v2 layout: activations arrive pre-transposed ([d, s] tiled) and pre-cast
(q/k fp8, v bf16) from the host, so the kernel does zero PE transposes.
Phase 2 folds Wo through kv (M = kv_bd @ WoT): the output projection consumes
exp(q_hat) directly; per-head normalization is applied to exp(q_hat) via a
PSUM broadcast matmul (sel2) before the projection.
"""

import functools
import numpy as np

B, S, D, H = 4, 4096, 1024, 16
DK = D // H          # 64
OG = D // 2          # 512 per-core head-group width (8 heads)
NCORES = 8
SCALE = 1.0 / 8.0    # 1/sqrt(DK)
NT = S // 128        # 32 s-tiles
SM = 512             # s-chunk per input DMA
NCH = S // SM        # 8 chunks
NU = SM // 128       # tiles per chunk


@functools.lru_cache(maxsize=2)
def _build(kv_bias=False):
    import concourse.bass as bass  # noqa: F401
    from concourse import bacc
    import concourse.mybir as mybir
    import concourse.tile as tile
    from contextlib import ExitStack

    f32 = mybir.dt.float32
    bf16 = mybir.dt.bfloat16
    fp8 = mybir.dt.float8e4
    DR = mybir.MatmulPerfMode.DoubleRow
    EXP = mybir.ActivationFunctionType.Exp
    COPY = mybir.ActivationFunctionType.Copy
    AXX = mybir.AxisListType.X
    ADD = mybir.AluOpType.add

    nc = bacc.Bacc()

    # x*[p, a, t, s'] = x[SM*a + s', 128*t + p] — per-partition-contiguous chunks
    xq = nc.declare_dram_parameter("xq", [128, NCH, 8, SM], fp8, isOutput=False)
    xk = nc.declare_dram_parameter("xk", [128, NCH, 8, SM], fp8, isOutput=False)
    xv = nc.declare_dram_parameter("xv", [128, NCH, 8, SM], bf16, isOutput=False)
    # w*[p, t, o] = W[og_slice, :].T[128*t + p, o]
    wq = nc.declare_dram_parameter("wq", [128, 8, OG], fp8, isOutput=False)
    wk = nc.declare_dram_parameter("wk", [128, 8, OG], fp8, isOutput=False)
    wv = nc.declare_dram_parameter("wv", [128, 8, OG], bf16, isOutput=False)
    # wo[p, t, d] = Wo[:, og_slice].T[128*t + p, d]
    wo = nc.declare_dram_parameter("wo", [128, 4, D], bf16, isOutput=False)
    bqsp = nc.declare_dram_parameter("bqs", [128, 4], f32, isOutput=False)
    bkp = nc.declare_dram_parameter("bk", [1, OG], f32, isOutput=False)
    bvp = nc.declare_dram_parameter("bv", [1, OG], f32, isOutput=False)
    maskp = nc.declare_dram_parameter("maskf", [128, NT], f32, isOutput=False)
    sel8p = nc.declare_dram_parameter("sel8", [8, 4, 128], bf16, isOutput=False)
    out = nc.declare_dram_parameter("out", [S, D], bf16, isOutput=True)

    with tile.TileContext(nc) as tc:
        with ExitStack() as ctx:
            singles = ctx.enter_context(tc.tile_pool(name="singles", bufs=1))

            # weight loads spread across queues, in need-order:
            # k/v first (first consumers), wo last (needed only at interphase)
            wk_sb = singles.tile([128, 8, OG], fp8, tag="wk")
            nc.scalar.dma_start(out=wk_sb, in_=wk[:, :, :])
            wv_sb = singles.tile([128, 8, OG], bf16, tag="wv")
            nc.scalar.dma_start(out=wv_sb, in_=wv[:, :, :])
            wq_sb = singles.tile([128, 8, OG], fp8, tag="wq")
            nc.scalar.dma_start(out=wq_sb, in_=wq[:, :, :])
            wo_sb = singles.tile([128, 4, D], bf16, tag="wo")
            nc.sync.dma_start(out=wo_sb, in_=wo[:, :, :])

            bqs_sb = singles.tile([128, 4], f32, tag="bqs")
            nc.scalar.dma_start(out=bqs_sb, in_=bqsp[:, :])
            if kv_bias:
                bk_bc = singles.tile([128, OG], f32, tag="bk_bc")
                nc.gpsimd.dma_start(out=bk_bc, in_=bkp[:, :].partition_broadcast(128))
                bv_bc = singles.tile([128, OG], f32, tag="bv_bc")
                nc.gpsimd.dma_start(out=bv_bc, in_=bvp[:, :].partition_broadcast(128))
            mask_sb = singles.tile([128, NT], f32, tag="mask")
            nc.scalar.dma_start(out=mask_sb, in_=maskp[:, :])

            # constants
            ones_s = singles.tile([128, 1], bf16, tag="ones_s")
            nc.vector.memset(ones_s, 1.0)
            ones1 = singles.tile([1, 1], f32, tag="ones1")
            nc.vector.memset(ones1, 1.0)
            # sel8[:, p, :]: partition j broadcasts 1/denom of head j to the
            # o-rows of pair p ([0:64] -> head 2p, [64:128] -> head 2p+1)
            sel8 = singles.tile([8, 4, 128], bf16, tag="sel8")
            nc.sync.dma_start(out=sel8, in_=sel8p[:, :, :])

            # exp(q_hat * scale), stored [o (4 blocks of 128 = head pairs), s]
            ET = singles.tile([128, 4, S], bf16, tag="ET")
            # fused (block-diag kv) @ WoT, plus per-pair head-slot ksum
            M_sb = singles.tile([128, 4, D], bf16, tag="M")
            # ksum_bd8[o', p, j] = ksum[o'] if head(p, o') == j else 0
            ksum_bd8 = singles.tile([128, 4, 8], bf16, tag="ksum_bd8")

            # ---------------- phase 1 ----------------
            with ExitStack() as p1:
                pacc_pool = p1.enter_context(tc.tile_pool(name="pacc", bufs=1, space="PSUM"))
                # kv^T accumulated per head pair: [e', pair, o'] (with cross-head
                # garbage at off-diagonal 64-blocks, masked out later)
                kvT_ps = pacc_pool.tile([128, 4, 128], f32, tag="kvT", name="kvT")
                # ksum^T accumulated per pair: [1, pair, o']
                ksT_ps = pacc_pool.tile([1, 4, 128], f32, tag="ksT", name="ksT")

                with ExitStack() as p1a:
                    xin_pool = p1a.enter_context(tc.tile_pool(name="xin", bufs=3))
                    kvf_pool = p1a.enter_context(tc.tile_pool(name="kvf", bufs=3))
                    pkv_pool = p1a.enter_context(tc.tile_pool(name="pkv", bufs=5, space="PSUM"))

                    pending = None  # (kf, vf, st) deferred kv accumulation

                    def flush_kv(pending):
                        kf, vf, pst = pending
                        for p in range(4):
                            nc.tensor.matmul(
                                kvT_ps[:, p, :],
                                vf[:, p, :],
                                kf[:, 2 * p:2 * p + 2, :],
                                start=(pst == 0 and p == 0),
                                stop=(pst == NT - 1),
                                skip_group_check=True,
                            )
                        for p in range(4):
                            nc.tensor.matmul(
                                ksT_ps[0:1, p, :],
                                ones_s,
                                kf[:, 2 * p:2 * p + 2, :],
                                start=(pst == 0 and p == 0),
                                stop=(pst == NT - 1),
                                skip_group_check=True,
                            )

                    for a in range(NCH):
                        xq_sb = xin_pool.tile([128, 8, SM], fp8, tag="xq")
                        nc.gpsimd.dma_start(out=xq_sb, in_=xq[:, a, :, :])
                        xk_sb = xin_pool.tile([128, 8, SM], fp8, tag="xk")
                        nc.gpsimd.dma_start(out=xk_sb, in_=xk[:, a, :, :])
                        xv_sb = xin_pool.tile([128, 8, SM], bf16, tag="xv")
                        nc.gpsimd.dma_start(out=xv_sb, in_=xv[:, a, :, :])

                        for u in range(NU):
                            st = a * NU + u
                            usl = slice(u * 128, (u + 1) * 128)

                            # k projection -> [s, og]
                            pk = pkv_pool.tile([128, OG], f32, tag="pkv")
                            for t2 in range(4):
                                nc.tensor.matmul(pk, xk_sb[:, 2 * t2:2 * t2 + 2, usl],
                                                 wk_sb[:, 2 * t2:2 * t2 + 2, :],
                                                 start=(t2 == 0), stop=(t2 == 3), perf_mode=DR)
                            if kv_bias:
                                nc.vector.tensor_add(pk, pk, bk_bc)
                            ek = kvf_pool.tile([128, OG], bf16, tag="ek")
                            nc.scalar.activation(ek, pk, EXP, scale=SCALE)
                            rows = kvf_pool.tile([128, 8], f32, tag="rows")
                            nc.vector.tensor_reduce(rows, ek.rearrange("p (h e) -> p h e", h=8), axis=AXX, op=ADD)
                            nc.vector.reciprocal(rows, rows)
                            nc.vector.tensor_scalar_mul(rows, rows, mask_sb[:, st:st + 1])
                            kf = kvf_pool.tile([128, 8, DK], bf16, tag="kf")
                            nc.vector.tensor_mul(
                                kf,
                                ek.rearrange("p (h e) -> p h e", h=8),
                                rows[:, :, None].to_broadcast([128, 8, DK]),
                            )

                            # v projection -> [s, og]
                            pv = pkv_pool.tile([128, OG], f32, tag="pkv")
                            for t in range(8):
                                nc.tensor.matmul(pv, xv_sb[:, t, usl], wv_sb[:, t, :],
                                                 start=(t == 0), stop=(t == 7))
                            if kv_bias:
                                nc.vector.tensor_add(pv, pv, bv_bc)
                            vf = kvf_pool.tile([128, 4, 128], bf16, tag="vf")
                            nc.scalar.activation(vf, pv.rearrange("p (j e) -> p j e", j=4),
                                                 COPY, scale=mask_sb[:, st:st + 1])

                            # deferred kv/ksum accumulation for the previous s-tile
                            if pending is not None:
                                flush_kv(pending)
                            pending = (kf, vf, st)

                        # q projection for the chunk, output transposed [o, s]
                        for ob in range(4):
                            pq = pkv_pool.tile([128, SM], f32, tag="pkv")
                            for t2 in range(4):
                                nc.tensor.matmul(pq, wq_sb[:, 2 * t2:2 * t2 + 2, ob * 128:(ob + 1) * 128],
                                                 xq_sb[:, 2 * t2:2 * t2 + 2, :],
                                                 start=(t2 == 0), stop=(t2 == 3), perf_mode=DR)
                            nc.scalar.activation(ET[:, ob, a * SM:(a + 1) * SM], pq, EXP,
                                                 bias=bqs_sb[:, ob:ob + 1], scale=SCALE)

                    flush_kv(pending)

                # ---------------- interphase ----------------
                pks_pool = p1.enter_context(tc.tile_pool(name="pks", bufs=1, space="PSUM"))
                pm_pool = p1.enter_context(tc.tile_pool(name="pm", bufs=2, space="PSUM"))

                # block-diagonal kv^T (zero the cross-head 64-blocks)
                kvbd = singles.tile([128, 4, 128], bf16, tag="kvbd")
                nc.vector.memset(kvbd, 0.0)
                for p in range(4):
                    nc.vector.tensor_copy(kvbd[0:64, p, 0:64], kvT_ps[0:64, p, 0:64])
                    nc.vector.tensor_copy(kvbd[64:128, p, 64:128], kvT_ps[64:128, p, 64:128])

                # transpose ksum^T [1, o'] -> [o', 1] via K=1 matmul, then
                # scatter into per-head columns of ksum_bd8
                ksT_sb = singles.tile([1, 4, 128], f32, tag="ksT_sb")
                nc.scalar.copy(out=ksT_sb, in_=ksT_ps)
                pks = pks_pool.tile([128, 4, 1], f32, tag="pks")
                for p in range(4):
                    nc.tensor.matmul(pks[:, p, :], ksT_sb[0:1, p, :], ones1, start=True, stop=True)
                nc.vector.memset(ksum_bd8, 0.0)
                for p in range(4):
                    nc.vector.tensor_copy(ksum_bd8[0:64, p, 2 * p:2 * p + 1], pks[0:64, p, :])
                    nc.vector.tensor_copy(ksum_bd8[64:128, p, 2 * p + 1:2 * p + 2], pks[64:128, p, :])

                # M = kv_bd @ WoT (per pair block)
                for p in range(4):
                    for half in range(2):
                        pm = pm_pool.tile([128, 512], f32, tag="pm")
                        nc.tensor.matmul(pm, kvbd[:, p, :], wo_sb[:, p, half * 512:(half + 1) * 512],
                                         start=True, stop=True)
                        nc.scalar.copy(out=M_sb[:, p, half * 512:(half + 1) * 512], in_=pm)

            # ---------------- phase 2 ----------------
            with ExitStack() as p2s:
                sb2 = p2s.enter_context(tc.tile_pool(name="sb2", bufs=3))
                pd_pool = p2s.enter_context(tc.tile_pool(name="pd", bufs=2, space="PSUM"))
                prd_pool = p2s.enter_context(tc.tile_pool(name="prd", bufs=2, space="PSUM"))
                pout_pool = p2s.enter_context(tc.tile_pool(name="pout", bufs=2, space="PSUM"))

                for st in range(NT):
                    s0 = st * 128
                    # per-head denominators accumulated into one [8, s'] tile:
                    # pd8[j, s'] = <ksum_(head j), ET_(head j)[:, s']>
                    pd8 = pd_pool.tile([8, 128], f32, tag="pd", name="pd")
                    for p in range(4):
                        nc.tensor.matmul(pd8, ksum_bd8[:, p, :], ET[:, p, s0:s0 + 128],
                                         start=(p == 0), stop=(p == 3))
                    rs_pre = sb2.tile([8, 128], f32, tag="rspre", name="rspre")
                    nc.scalar.activation(rs_pre, pd8, COPY, bias=1e-6)
                    rs8 = sb2.tile([8, 128], bf16, tag="rs", name="rs")
                    with nc.allow_low_precision(reason="1/denom in bf16 is within tolerance"):
                        nc.vector.reciprocal(rs8, rs_pre)
                    # broadcast 1/denom to the 64 o-rows of each head
                    prd = prd_pool.tile([128, 4, 128], f32, tag="prd", name="prd")
                    for p in range(4):
                        nc.tensor.matmul(prd[:, p, :], sel8[:, p, :], rs8, start=True, stop=True)
                    # normalized exp(q) features
                    ets = sb2.tile([128, 4, 128], bf16, tag="ets", name="ets")
                    nc.vector.tensor_mul(ets, ET[:, :, s0:s0 + 128], prd)
                    # fused output projection
                    pout = pout_pool.tile([128, 2, 512], f32, tag="pout", name="pout")
                    for half in range(2):
                        for p in range(4):
                            nc.tensor.matmul(pout[:, half, :], ets[:, p, :],
                                             M_sb[:, p, half * 512:(half + 1) * 512],
                                             start=(p == 0), stop=(p == 3))
                    outsb = sb2.tile([128, D], bf16, tag="outsb", name="outsb")
                    nc.scalar.copy(out=outsb.rearrange("p (j e) -> p j e", j=2), in_=pout)
                    nc.sync.dma_start(out=out[s0:s0 + 128, :], in_=outsb)

    nc.compile()
    return nc


_LAST_RESULT = None


def _ensure_ntff_hook():
    """Make `antenv.axon_hooks` importable so BASS_TRACE profiling works.

    Some images ship a minimal `antenv` stub without `axon_hooks`; the boot
    shim then degrades silently and bass_utils crashes on import when
    trace=True under axon. Inject the module and install the ctypes NTFF
    hook if possible. No-op when the real module exists.
    """
    try:
        from antenv import axon_hooks  # noqa: F401
        return
    except ImportError:
        pass
    import sys
    import types
    try:
        import antenv
    except ImportError:
        return
    mod = types.ModuleType("antenv.axon_hooks")
    mod._hook = None

    def set_axon_ntff_profile_hook(hook):
        mod._hook = hook

    def get_axon_ntff_profile_hook():
        return mod._hook

    mod.set_axon_ntff_profile_hook = set_axon_ntff_profile_hook
    mod.get_axon_ntff_profile_hook = get_axon_ntff_profile_hook
    sys.modules["antenv.axon_hooks"] = mod
    antenv.axon_hooks = mod
    try:
        from trn_agent_boot.trn_boot import _ntff_profile_via_ctypes

        hook = _ntff_profile_via_ctypes("/opt/axon/libaxon_pjrt.so")
        if hook is not None:
            set_axon_ntff_profile_hook(hook)
    except Exception:
        pass


def kernel(q, k, v, mask, Wq, bq, Wk, bk, Wv, bv, Wo, bo):
    global _LAST_RESULT
    import ml_dtypes
    from concourse.bass_utils import run_bass_kernel_spmd

    _ensure_ntff_hook()

    q = np.asarray(q, np.float32)
    k = np.asarray(k, np.float32)
    v = np.asarray(v, np.float32)
    mask = np.asarray(mask)
    Wq = np.asarray(Wq, np.float32)
    Wk = np.asarray(Wk, np.float32)
    Wv = np.asarray(Wv, np.float32)
    Wo = np.asarray(Wo, np.float32)
    bq = np.asarray(bq, np.float32)
    bk = np.asarray(bk, np.float32)
    bv = np.asarray(bv, np.float32)
    bo = np.asarray(bo, np.float32)

    nc = _build(bool(np.any(bk) or np.any(bv)))

    bf = ml_dtypes.bfloat16
    f8 = ml_dtypes.float8_e4m3

    def xtile(x, dt):
        # [S, D] -> [128, NCH, 8, SM]: A[p, a, t, s'] = x[SM*a + s', 128*t + p]
        xt = np.ascontiguousarray(x.T)
        return xt.reshape(8, 128, NCH, SM).transpose(1, 2, 0, 3).astype(dt)

    def wtile(W, sl, nt, dt):
        # [128, nt, ncols]: w[p, t, o] = W[sl, :].T[128*t + p, o]
        wt = np.ascontiguousarray(W[sl, :].T) if sl is not None else W
        return wt.reshape(nt, 128, -1).transpose(1, 0, 2).astype(dt)

    sel8_host = np.zeros((8, 4, 128), bf)
    for p in range(4):
        sel8_host[2 * p, p, 0:64] = 1
        sel8_host[2 * p + 1, p, 64:128] = 1

    in_maps = []
    xcache = {}
    for core in range(NCORES):
        b, g = core // 2, core % 2
        sl = slice(g * OG, (g + 1) * OG)
        if b not in xcache:
            xcache[b] = (xtile(q[b], f8), xtile(k[b], f8), xtile(v[b], bf))
        xqh, xkh, xvh = xcache[b]
        maskf = mask[b, 0, 0, :].astype(np.float32).reshape(NT, 128).T.copy()
        in_maps.append({
            "xq": xqh,
            "xk": xkh,
            "xv": xvh,
            "wq": wtile(Wq, sl, 8, f8),
            "wk": wtile(Wk, sl, 8, f8),
            "wv": wtile(Wv, sl, 8, bf),
            "wo": wtile(np.ascontiguousarray(Wo[:, sl].T), None, 4, bf),
            "bqs": np.ascontiguousarray((bq[sl] * SCALE).reshape(4, 128).T),
            "bk": bk[sl].reshape(1, OG).copy(),
            "bv": bv[sl].reshape(1, OG).copy(),
            "maskf": maskf,
            "sel8": sel8_host,
        })

    res = run_bass_kernel_spmd(nc, in_maps, list(range(NCORES)))
    _LAST_RESULT = res

    outp = np.empty((B, S, D), np.float32)
    for b in range(B):
        outp[b] = (res.results[2 * b]["out"].astype(np.float32)
                   + res.results[2 * b + 1]["out"].astype(np.float32)
                   + bo[None, :])
    return outp
